# revision 1
# baseline (speedup 1.0000x reference)
"""AdapterFusionBlock Trainium2 kernel: 8-way batch-parallel, one sample per core.

Self-contained: hardcodes all shapes. Host folds LN affines + adapter scale +
attention scale into weights; window-permutes tokens. Per-core Bass/Tile graph:
LN -> qkv (+fp8 DoubleRow adapter) -> windowed attention in S^T orientation
(softmax denominators via a ones-column in V; token-major PV; rel-pos bias via
shift-gather + indicator matmuls accumulated into S^T) -> proj -> residual ->
inline per-tile LN2 -> MLP interleaved chunk-by-chunk with attention windows.
"""
import sys
sys.path.insert(0, '/opt/trn_rl_repo')
import numpy as np
import ml_dtypes
import concourse.bass as bass
import concourse.mybir as mybir
import concourse.tile as tile
from concourse import bacc
from concourse.bass_utils import run_bass_kernel_spmd
from concourse.masks import make_identity

FP32 = mybir.dt.float32
BF16 = mybir.dt.bfloat16
FP8 = mybir.dt.float8e4
AF = mybir.ActivationFunctionType
ALU = mybir.AluOpType
DR = mybir.MatmulPerfMode.DoubleRow

DIM = 768; NH = 12; HD = 64; WS = 16; B = 8; H = 64; W = 64
MLPD = 4 * DIM; AD = 3 * DIM // 4; HID = DIM // 2
BLOCK_SCALE = 0.5; EPS = 1e-5
T = H * W                  # 4096 tokens per core
NWIN = (H // WS) * (W // WS)   # 16 windows
NT = WS * WS               # 256 tokens per window
CH = 512                   # token chunk for GEMM phases
NCH = T // CH              # 8
SCALE = HD ** -0.5         # 0.125
AD8 = 640                  # adapter hidden padded to 5*128
VR = NH * 65               # 780 v2 rows (64 d + 1 ones per head)
VRP = 784                  # padded to mult of 16 for transpose-DMA

_BF = ml_dtypes.bfloat16
_F8 = ml_dtypes.float8_e4m3


def _bf16(x):
    return np.ascontiguousarray(x.astype(_BF))


def _f8(x):
    return np.ascontiguousarray(np.clip(x, -240, 240).astype(_F8))


def _col_tiles(v):
    """[n*128] -> [128, n] column layout (col k = channels k*128..k*128+127)."""
    n = v.shape[0] // 128
    return np.ascontiguousarray(v.reshape(n, 128).T.astype(np.float32))


def build_graph():
    nc = bacc.Bacc()
    P = 128

    # ---------------- DRAM parameters ----------------
    x_in = nc.declare_dram_parameter("x", [T, DIM], FP32, isOutput=False)
    wqkv = nc.declare_dram_parameter("wqkv", [DIM, 3 * DIM], BF16, isOutput=False)
    a1w8 = nc.declare_dram_parameter("a1w8", [9 * P, 2 * AD8], FP8, isOutput=False)
    a2w8 = nc.declare_dram_parameter("a2w8", [2 * P, 2 * 3 * DIM], FP8, isOutput=False)
    a2wl = nc.declare_dram_parameter("a2wl", [P, 3 * DIM], FP8, isOutput=False)
    wp = nc.declare_dram_parameter("wp", [DIM, DIM], BF16, isOutput=False)
    wm1 = nc.declare_dram_parameter("wm1", [DIM, MLPD], BF16, isOutput=False)
    wm2 = nc.declare_dram_parameter("wm2", [MLPD, DIM], BF16, isOutput=False)
    relcatT = nc.declare_dram_parameter("relcatT", [HD, 62], BF16, isOutput=False)
    indic96 = nc.declare_dram_parameter("indic96", [96, NT], BF16, isOutput=False)
    bqkv_c = nc.declare_dram_parameter("bqkv_c", [P, 18], FP32, isOutput=False)
    ba1_c = nc.declare_dram_parameter("ba1_c", [P, 5], FP32, isOutput=False)
    ba2_c = nc.declare_dram_parameter("ba2_c", [P, 18], FP32, isOutput=False)
    bm1_c = nc.declare_dram_parameter("bm1_c", [P, 24], FP32, isOutput=False)
    bp_r = nc.declare_dram_parameter("bp_r", [1, DIM], BF16, isOutput=False)
    bm2_r = nc.declare_dram_parameter("bm2_r", [1, DIM], BF16, isOutput=False)
    w1_c = nc.declare_dram_parameter("w1_c", [P, 6], FP32, isOutput=False)
    b1_c = nc.declare_dram_parameter("b1_c", [P, 6], FP32, isOutput=False)
    w1_r = nc.declare_dram_parameter("w1_r", [1, DIM], FP32, isOutput=False)
    b1_r = nc.declare_dram_parameter("b1_r", [1, DIM], FP32, isOutput=False)
    cw_r = nc.declare_dram_parameter("cw_r", [1, HID], FP32, isOutput=False)
    cb_r = nc.declare_dram_parameter("cb_r", [1, HID], FP32, isOutput=False)
    sw_r = nc.declare_dram_parameter("sw_r", [1, HID], FP32, isOutput=False)
    sb_r = nc.declare_dram_parameter("sb_r", [1, HID], FP32, isOutput=False)
    out_ext = nc.declare_dram_parameter("out", [T, DIM], FP32, isOutput=True)
    import os
    KDBG = os.environ.get("KDBG", "0") == "1"
    if KDBG:
        dbgq = nc.declare_dram_parameter("dbgq", [DIM, T], BF16, isOutput=True)
        dbgk = nc.declare_dram_parameter("dbgk", [DIM, T], BF16, isOutput=True)
        dbgv = nc.declare_dram_parameter("dbgv", [VRP, T], BF16, isOutput=True)
        dbgad = nc.declare_dram_parameter("dbgad", [DIM, T], BF16, isOutput=True)
        dbgn1 = nc.declare_dram_parameter("dbgn1", [T, DIM], BF16, isOutput=True)
        dbgn2 = nc.declare_dram_parameter("dbgn2", [T, DIM], BF16, isOutput=True)

    NTILES = T // P  # 32

    with tile.TileContext(nc) as tc:
        with tc.tile_pool(name="const", bufs=1) as const, \
             tc.tile_pool(name="stats", bufs=1) as stats, \
             tc.tile_pool(name="ntc", bufs=6) as ntcp, \
             tc.tile_pool(name="dram", bufs=1, space="DRAM") as dramp, \
             tc.tile_pool(name="w2b", bufs=1) as w2b:
            # ---- DRAM scratch (pool tiles => dependency-tracked) ----
            norm_d = dramp.tile([T, DIM], BF16, name="norm_d")
            q_d = dramp.tile([DIM, T], BF16, name="q_d")
            k_d = dramp.tile([DIM, T], BF16, name="k_d")
            v2_d = dramp.tile([VRP, T], BF16, name="v2_d")
            ad_d = dramp.tile([DIM, T], BF16, name="ad_d")
            norm2_d = dramp.tile([T, DIM], BF16, name="norm2_d")
            t_dH = dramp.tile([32, P, 768], BF16, name="t_dH")
            t_dW = dramp.tile([32, P, 768], BF16, name="t_dW")
            vec_d = dramp.tile([4, HID], FP32, name="vec_d")
            ident = const.tile([P, P], BF16)
            make_identity(nc, ident[:, :])
            ones1 = const.tile([1, P], BF16)
            nc.vector.memset(ones1[:, :], 1.0)
            ones128 = const.tile([P, 1], BF16)
            nc.vector.memset(ones128[:, :], 1.0)
            eps_col = const.tile([P, 1], FP32)
            nc.vector.memset(eps_col[:, :], EPS)
            relcatT_sb = const.tile([HD, 62], BF16)
            nc.sync.dma_start(out=relcatT_sb[:, :], in_=relcatT[:, :])
            indic_sb = const.tile([96, NT], BF16)
            nc.sync.dma_start(out=indic_sb[:, :], in_=indic96[:, :])
            bp_sb = const.tile([1, DIM], BF16)
            nc.sync.dma_start(out=bp_sb[:, :], in_=bp_r[:, :])
            bm2_sb = const.tile([1, DIM], BF16)
            nc.sync.dma_start(out=bm2_sb[:, :], in_=bm2_r[:, :])
            bqkv_sb = const.tile([P, 18], FP32)
            nc.sync.dma_start(out=bqkv_sb[:, :], in_=bqkv_c[:, :])
            ba1_sb = const.tile([P, 5], FP32)
            nc.sync.dma_start(out=ba1_sb[:, :], in_=ba1_c[:, :])
            ba2_sb = const.tile([P, 18], FP32)
            nc.sync.dma_start(out=ba2_sb[:, :], in_=ba2_c[:, :])
            bm1_sb = const.tile([P, 24], FP32)
            nc.sync.dma_start(out=bm1_sb[:, :], in_=bm1_c[:, :])
            w1c_sb = const.tile([P, 6], FP32)
            nc.sync.dma_start(out=w1c_sb[:, :], in_=w1_c[:, :])
            b1c_sb = const.tile([P, 6], FP32)
            nc.sync.dma_start(out=b1c_sb[:, :], in_=b1_c[:, :])
            w1r_sb = const.tile([1, DIM], FP32)
            nc.sync.dma_start(out=w1r_sb[:, :], in_=w1_r[:, :])
            b1r_sb = const.tile([1, DIM], FP32)
            nc.sync.dma_start(out=b1r_sb[:, :], in_=b1_r[:, :])
            cw_sb = const.tile([1, HID], FP32)
            nc.sync.dma_start(out=cw_sb[:, :], in_=cw_r[:, :])
            cb_sb = const.tile([1, HID], FP32)
            nc.sync.dma_start(out=cb_sb[:, :], in_=cb_r[:, :])
            sw_sb = const.tile([1, HID], FP32)
            nc.sync.dma_start(out=sw_sb[:, :], in_=sw_r[:, :])
            sb_sb = const.tile([1, HID], FP32)
            nc.sync.dma_start(out=sb_sb[:, :], in_=sb_r[:, :])
            # proj weights (tiny; load from t=0 into their own region)
            wp_sb = [w2b.tile([P, DIM], BF16, tag="wp", bufs=6, name=f"wp{_i}")
                     for _i in range(6)]
            for k in range(6):
                nc.scalar.dma_start(out=wp_sb[k][:, :], in_=wp[k * P:(k + 1) * P, :])

            # ============ PHASE 1: LN1 + norm + adapter sums ============
            with tc.tile_pool(name="p1", bufs=4) as p1, \
                 tc.tile_pool(name="p1ps", bufs=1, space="PSUM") as p1ps:
                sum_ps = p1ps.tile([1, DIM], FP32)    # sum_tok(norm)
                sq_ps = p1ps.tile([1, DIM], FP32)     # sum_tok(norm^2)
                # ones rows of v2_d (col 64 of each head's v-block)
                ones12 = p1.tile([16, T], BF16, tag="ones12", bufs=1)
                nc.vector.memset(ones12[:, :], 1.0)
                for h_ in range(NH):
                    nc.sync.dma_start(out=v2_d[h_ * 65 + 64:h_ * 65 + 65, :],
                                      in_=ones12[h_:h_ + 1, :])
                for t in range(NTILES):
                    xt = p1.tile([P, DIM], FP32, tag="xt", bufs=4)
                    nc.sync.dma_start(out=xt[:, :], in_=x_in[t * P:(t + 1) * P, :])
                    sm = p1.tile([P, 1], FP32, tag="sm", bufs=4)
                    nc.vector.tensor_reduce(sm[:, :], xt[:, :],
                                            axis=mybir.AxisListType.X, op=ALU.add)
                    scr = p1.tile([P, DIM], BF16, tag="scr", bufs=4)
                    sq = p1.tile([P, 1], FP32, tag="sq", bufs=4)
                    nc.scalar.activation(scr[:, :], xt[:, :], AF.Square,
                                         accum_out=sq[:, :])
                    mean = p1.tile([P, 1], FP32, tag="mean", bufs=4)
                    nc.vector.tensor_scalar(mean[:, :], sm[:, :], 1.0 / DIM, None, op0=ALU.mult)
                    var = p1.tile([P, 1], FP32, tag="var", bufs=4)
                    nc.vector.tensor_scalar(var[:, :], sq[:, :], 1.0 / DIM, None, op0=ALU.mult)
                    m2c = p1.tile([P, 1], FP32, tag="m2c", bufs=4)
                    nc.vector.tensor_tensor(m2c[:, :], mean[:, :], mean[:, :], op=ALU.mult)
                    nc.vector.tensor_tensor(var[:, :], var[:, :], m2c[:, :], op=ALU.subtract)
                    sdv = p1.tile([P, 1], FP32, tag="sdv", bufs=4)
                    nc.scalar.activation(sdv[:, :], var[:, :], AF.Ln, bias=eps_col[:, :])
                    rstd = p1.tile([P, 1], FP32, tag="rstd", bufs=4)
                    nc.scalar.activation(rstd[:, :], sdv[:, :], AF.Exp, scale=-0.5)
                    nt = p1.tile([P, DIM], BF16, tag="nt", bufs=4)
                    nc.vector.tensor_scalar(nt[:, :], xt[:, :], mean[:, :],
                                            rstd[:, :], op0=ALU.subtract, op1=ALU.mult)
                    nc.scalar.dma_start(out=norm_d[t * P:(t + 1) * P, :], in_=nt[:, :])
                    nsq = p1.tile([P, DIM], BF16, tag="nsq", bufs=4)
                    nc.scalar.activation(nsq[:, :], nt[:, :], AF.Square)
                    for n2 in range(2):
                        sl = slice(n2 * 384, (n2 + 1) * 384)
                        nc.tensor.matmul(sum_ps[:, sl], ones128[:, :], nt[:, sl],
                                         start=(t == 0), stop=(t == NTILES - 1))
                        nc.tensor.matmul(sq_ps[:, sl], ones128[:, :], nsq[:, sl],
                                         start=(t == 0), stop=(t == NTILES - 1))
                # adapter per-channel math (rows [1, *])
                Mn = stats.tile([1, DIM], FP32)
                nc.vector.tensor_scalar(Mn[:, :], sum_ps[:, :], 1.0 / T, None, op0=ALU.mult)
                Sq = stats.tile([1, DIM], FP32)
                nc.vector.tensor_scalar(Sq[:, :], sq_ps[:, :], 1.0 / T, None, op0=ALU.mult)
                mch = stats.tile([1, HID], FP32)
                nc.vector.tensor_tensor(mch[:, :], w1r_sb[:, 0:HID], Mn[:, 0:HID], op=ALU.mult)
                nc.vector.tensor_tensor(mch[:, :], mch[:, :], b1r_sb[:, 0:HID], op=ALU.add)
                sig_in = stats.tile([1, HID], FP32)
                nc.vector.tensor_tensor(sig_in[:, :], cw_sb[:, :], mch[:, :], op=ALU.mult)
                nc.vector.tensor_tensor(sig_in[:, :], sig_in[:, :], cb_sb[:, :], op=ALU.add)
                s0 = stats.tile([1, HID], FP32)
                nc.scalar.activation(s0[:, :], sig_in[:, :], AF.Sigmoid)
                g0 = stats.tile([1, HID], FP32)
                nc.vector.tensor_tensor(g0[:, :], w1r_sb[:, 0:HID], s0[:, :], op=ALU.mult)
                h0 = stats.tile([1, HID], FP32)
                nc.vector.tensor_tensor(h0[:, :], b1r_sb[:, 0:HID], s0[:, :], op=ALU.mult)
                u = stats.tile([1, HID], FP32)
                nc.vector.tensor_tensor(u[:, :], w1r_sb[:, HID:DIM], Mn[:, HID:DIM], op=ALU.mult)
                nc.vector.tensor_tensor(u[:, :], u[:, :], b1r_sb[:, HID:DIM], op=ALU.add)
                mu = stats.tile([1, 1], FP32)
                nc.vector.tensor_reduce(mu[:, :], u[:, :], axis=mybir.AxisListType.X, op=ALU.add)
                nc.vector.tensor_scalar(mu[:, :], mu[:, :], 1.0 / HID, None, op0=ALU.mult)
                e1 = stats.tile([1, HID], FP32)
                nc.vector.tensor_tensor(e1[:, :], w1r_sb[:, HID:DIM], w1r_sb[:, HID:DIM], op=ALU.mult)
                nc.vector.tensor_tensor(e1[:, :], e1[:, :], Sq[:, HID:DIM], op=ALU.mult)
                e2 = stats.tile([1, HID], FP32)
                nc.vector.tensor_tensor(e2[:, :], w1r_sb[:, HID:DIM], b1r_sb[:, HID:DIM], op=ALU.mult)
                nc.vector.tensor_tensor(e2[:, :], e2[:, :], Mn[:, HID:DIM], op=ALU.mult)
                nc.vector.tensor_scalar(e2[:, :], e2[:, :], 2.0, None, op0=ALU.mult)
                nc.vector.tensor_tensor(e1[:, :], e1[:, :], e2[:, :], op=ALU.add)
                e3 = stats.tile([1, HID], FP32)
                nc.vector.tensor_tensor(e3[:, :], b1r_sb[:, HID:DIM], b1r_sb[:, HID:DIM], op=ALU.mult)
                nc.vector.tensor_tensor(e1[:, :], e1[:, :], e3[:, :], op=ALU.add)
                E2 = stats.tile([1, 1], FP32)
                nc.vector.tensor_reduce(E2[:, :], e1[:, :], axis=mybir.AxisListType.X, op=ALU.add)
                nc.vector.tensor_scalar(E2[:, :], E2[:, :], 1.0 / HID, None, op0=ALU.mult)
                mu2 = stats.tile([1, 1], FP32)
                nc.vector.tensor_tensor(mu2[:, :], mu[:, :], mu[:, :], op=ALU.mult)
                nc.vector.tensor_tensor(E2[:, :], E2[:, :], mu2[:, :], op=ALU.subtract)
                rv = stats.tile([1, 1], FP32)
                nc.scalar.activation(rv[:, :], E2[:, :], AF.Sqrt, bias=eps_col[0:1, :])
                nc.vector.reciprocal(rv[:, :], rv[:, :])
                Pv = stats.tile([1, HID], FP32)
                nc.vector.tensor_tensor(Pv[:, :], sw_sb[:, :], w1r_sb[:, HID:DIM], op=ALU.mult)
                nc.vector.tensor_scalar(Pv[:, :], Pv[:, :], rv[:, :], None, op0=ALU.mult)
                Qv = stats.tile([1, HID], FP32)
                nc.vector.tensor_scalar(Qv[:, :], b1r_sb[:, HID:DIM], mu[:, :], None, op0=ALU.subtract)
                nc.vector.tensor_tensor(Qv[:, :], Qv[:, :], sw_sb[:, :], op=ALU.mult)
                nc.vector.tensor_scalar(Qv[:, :], Qv[:, :], rv[:, :], None, op0=ALU.mult)
                nc.vector.tensor_tensor(Qv[:, :], Qv[:, :], sb_sb[:, :], op=ALU.add)
                nc.sync.dma_start(out=vec_d[0:1, :], in_=g0[0:1, :])
                nc.sync.dma_start(out=vec_d[1:2, :], in_=h0[0:1, :])
                nc.sync.dma_start(out=vec_d[2:3, :], in_=Pv[0:1, :])
                nc.sync.dma_start(out=vec_d[3:4, :], in_=Qv[0:1, :])
                g0c = stats.tile([P, 3], FP32); h0c = stats.tile([P, 3], FP32)
                Pc = stats.tile([P, 3], FP32); Qc = stats.tile([P, 3], FP32)
                for dst, row in ((g0c, 0), (h0c, 1), (Pc, 2), (Qc, 3)):
                    for kk in range(3):
                        src = bass.AP(tensor=vec_d[:, :].tensor,
                                      offset=vec_d[:, :].offset + row * HID + kk * P,
                                      ap=[[1, P], [1, 1]])
                        nc.sync.dma_start(out=dst[:, kk:kk + 1], in_=src)

            # ============ PHASE 2a: qkv + fp8 adapter (+ shuffle adapter) ============
            with tc.tile_pool(name="w2a", bufs=1) as w2a, \
                 tc.tile_pool(name="p2a", bufs=2) as p2a, \
                 tc.tile_pool(name="qk2a", bufs=19) as qk2a, \
                 tc.tile_pool(name="f8p", bufs=2) as f8p, \
                 tc.tile_pool(name="p3", bufs=3) as p3, \
                 tc.tile_pool(name="ps2a", bufs=4, space="PSUM") as ps2a:
                wqkv_sb = [w2a.tile([P, 3 * DIM], BF16, tag="wqkv", bufs=6, name=f"wqkv{_i}") for _i in range(6)]
                for k in range(6):
                    nc.scalar.dma_start(out=wqkv_sb[k][:, :], in_=wqkv[k * P:(k + 1) * P, :])
                a1p = [w2a.tile([P, 2 * AD8], FP8, tag="a1p", bufs=9, name=f"a1p{_i}") for _i in range(9)]
                for j in range(9):
                    nc.scalar.dma_start(out=a1p[j][:, :], in_=a1w8[j * P:(j + 1) * P, :])
                a2p = [w2a.tile([P, 2 * 3 * DIM], FP8, tag="a2p", bufs=2, name=f"a2p{_i}") for _i in range(2)]
                for j in range(2):
                    nc.scalar.dma_start(out=a2p[j][:, :], in_=a2w8[j * P:(j + 1) * P, :])
                a2l = w2a.tile([P, 3 * DIM], FP8, tag="a2l", bufs=1)
                nc.scalar.dma_start(out=a2l[:, :], in_=a2wl[:, :])
                for c in range(NCH):
                    csl = slice(c * CH, (c + 1) * CH)
                    ntc = [ntcp.tile([P, CH], BF16, tag="ntc", bufs=6, name=f"ntc{_i}") for _i in range(6)]
                    for k in range(6):
                        nc.sync.dma_start(out=ntc[k][:, :], in_=norm_d[csl, k * P:(k + 1) * P],
                                          transpose=True)
                    qkv8 = f8p.tile([P, 18 * CH], FP8, tag="qkv8", bufs=2)
                    qkvT = [qk2a.tile([P, CH], BF16, tag="qkvT", bufs=19, name=f"qkvT{_i}") for _i in range(18)]
                    for m in range(18):
                        ps = ps2a.tile([P, CH], FP32, tag="mm")
                        for k in range(6):
                            nc.tensor.matmul(ps[:, :], wqkv_sb[k][:, m * P:(m + 1) * P],
                                             ntc[k][:, :], start=(k == 0), stop=(k == 5))
                        nc.scalar.activation(qkvT[m][:, :], ps[:, :], AF.Identity,
                                             bias=bqkv_sb[:, m:m + 1])
                        nc.scalar.activation(qkv8[:, m * CH:(m + 1) * CH], ps[:, :],
                                             AF.Identity, bias=bqkv_sb[:, m:m + 1])
                    # adapter A1 (fp8 DoubleRow, contraction 2304 = 9 pairs)
                    ad18 = f8p.tile([P, 5 * CH], FP8, tag="ad18", bufs=2)
                    for m in range(5):
                        ps = ps2a.tile([P, CH], FP32, tag="mm")
                        for j in range(9):
                            lhs = bass.AP(tensor=a1p[j].tensor,
                                          offset=a1p[j][:, :].offset + m * P,
                                          ap=[[a1p[j].tensor.shape[1], P], [AD8, 2], [1, P]])
                            rhs = bass.AP(tensor=qkv8.tensor,
                                          offset=qkv8[:, :].offset + j * 2 * CH,
                                          ap=[[qkv8.tensor.shape[1], P], [CH, 2], [1, CH]])
                            nc.tensor.matmul(ps[:, :], lhs, rhs, start=(j == 0), stop=(j == 8),
                                             perf_mode=DR)
                        nc.scalar.activation(ad18[:, m * CH:(m + 1) * CH], ps[:, :], AF.Gelu,
                                             bias=ba1_sb[:, m:m + 1])
                    # adapter A2 (2 fp8 DoubleRow pairs + 1 normal) + combine + store
                    for m in range(18):
                        ps = ps2a.tile([P, CH], FP32, tag="mm")
                        for j in range(2):
                            lhs = bass.AP(tensor=a2p[j].tensor,
                                          offset=a2p[j][:, :].offset + m * P,
                                          ap=[[a2p[j].tensor.shape[1], P], [3 * DIM, 2], [1, P]])
                            rhs = bass.AP(tensor=ad18.tensor,
                                          offset=ad18[:, :].offset + j * 2 * CH,
                                          ap=[[ad18.tensor.shape[1], P], [CH, 2], [1, CH]])
                            nc.tensor.matmul(ps[:, :], lhs, rhs, start=(j == 0), stop=False,
                                             perf_mode=DR)
                        nc.tensor.matmul(ps[:, :], a2l[:, m * P:(m + 1) * P],
                                         ad18[:, 4 * CH:5 * CH], start=False, stop=True)
                        fin = p2a.tile([P, CH], BF16, tag="fin")
                        nc.vector.scalar_tensor_tensor(fin[:, :], ps[:, :], ba2_sb[:, m:m + 1],
                                                       qkvT[m][:, :], op0=ALU.add, op1=ALU.add)
                        if m < 12:
                            dst = (q_d, k_d)[m // 6]
                            nc.sync.dma_start(out=dst[(m % 6) * P:(m % 6 + 1) * P, csl],
                                              in_=fin[:, :])
                        else:
                            # v: scatter into v2_d rows h*65+d (2 heads per tile)
                            h0_ = 2 * (m - 12)
                            for hh_ in range(2):
                                r0 = (h0_ + hh_) * 65
                                nc.sync.dma_start(out=v2_d[r0:r0 + 64, csl],
                                                  in_=fin[hh_ * 64:(hh_ + 1) * 64, :])
                    # shuffle-adapter elementwise for this chunk
                    for pt in range(3):
                        a0 = p3.tile([P, CH], BF16, tag="a0")
                        nc.vector.tensor_scalar(a0[:, :], ntc[pt][:, :], g0c[:, pt:pt + 1],
                                                h0c[:, pt:pt + 1], op0=ALU.mult, op1=ALU.add)
                        nc.sync.dma_start(out=ad_d[pt * P:(pt + 1) * P, csl], in_=a0[:, :])
                        s1t = p3.tile([P, CH], BF16, tag="s1")
                        nc.scalar.activation(s1t[:, :], ntc[3 + pt][:, :], AF.Sigmoid,
                                             bias=Qc[:, pt:pt + 1], scale=Pc[:, pt:pt + 1])
                        t1 = p3.tile([P, CH], BF16, tag="t1")
                        nc.vector.tensor_scalar(t1[:, :], ntc[3 + pt][:, :], w1c_sb[:, 3 + pt:4 + pt],
                                                b1c_sb[:, 3 + pt:4 + pt], op0=ALU.mult, op1=ALU.add)
                        xs = p3.tile([P, CH], BF16, tag="xs")
                        nc.vector.tensor_tensor(xs[:, :], t1[:, :], s1t[:, :], op=ALU.mult)
                        nc.sync.dma_start(out=ad_d[HID + pt * P:HID + (pt + 1) * P, csl], in_=xs[:, :])

            # ============ PHASE 2b + 5: windowed attention + MLP interleaved ============
            with tc.tile_pool(name="w5", bufs=1) as w5, \
                 tc.tile_pool(name="p2b", bufs=2) as p2b, \
                 tc.tile_pool(name="xm", bufs=6) as xmp, \
                 tc.tile_pool(name="h5", bufs=25) as h5, \
                 tc.tile_pool(name="ps", bufs=1, space="PSUM") as psp:
                wm1_sb = [w5.tile([P, MLPD], BF16, tag="wm1", bufs=6, name=f"wm1_{_i}") for _i in range(6)]
                for k in range(6):
                    nc.scalar.dma_start(out=wm1_sb[k][:, :], in_=wm1[k * P:(k + 1) * P, :])
                wm2_sb = [w5.tile([P, DIM], BF16, tag="wm2", bufs=24, name=f"wm2_{_i}") for _i in range(24)]
                for k in range(24):
                    nc.scalar.dma_start(out=wm2_sb[k][:, :], in_=wm2[k * P:(k + 1) * P, :])
                xm_tiles = []
                for w in range(NWIN):
                    if True:
                        k_sb = p2b.tile([HD, NH * NT], BF16, tag="k", bufs=2)
                        src = bass.AP(tensor=k_d[:, :].tensor, offset=w * NT,
                                      ap=[[T, HD], [HD * T, NH], [1, NT]])
                        nc.sync.dma_start(out=k_sb[:, :], in_=src)
                        v_sb = [p2b.tile([P, VRP], BF16, tag="v", bufs=3, name=f"v{w}_{_i}")
                                for _i in range(2)]
                        for kc in range(2):
                            nc.sync.dma_start(out=v_sb[kc][:, :],
                                              in_=v2_d[:, w * NT + kc * P: w * NT + (kc + 1) * P],
                                              transpose=True)
                        for qt in range(2):
                            qtw = w * 2 + qt
                            q_sb = p2b.tile([HD, NH * P], BF16, tag="q", bufs=2)
                            src = bass.AP(tensor=q_d[:, :].tensor, offset=w * NT + qt * P,
                                          ap=[[T, HD], [HD * T, NH], [1, P]])
                            nc.sync.dma_start(out=q_sb[:, :], in_=src)
                            # --- T matmuls: [128q, (h, 62)] ---
                            t_ps = psp.tile([P, 744], FP32, tag="big", bufs=2)
                            for h in range(NH):
                                nc.tensor.matmul(t_ps[:, h * 62:(h + 1) * 62],
                                                 q_sb[:, h * P:(h + 1) * P],
                                                 relcatT_sb[:, :], start=True, stop=True)
                            t_sb = p2b.tile([P, 744], BF16, tag="tsb", bufs=3)
                            nc.vector.tensor_copy(t_sb[:, :], t_ps[:, :])
                            # --- write t twice with the shift folded into the dst AP ---
                            # t_dH[p, j - qh(p) + 16] = t[p, j];  t_dW[p, j - qw(p) + 16] = t[p, j]
                            hofs = t_dH[:, :, :].offset + qtw * P * 768
                            wofs = t_dW[:, :, :].offset + qtw * P * 768
                            dstH1 = bass.AP(tensor=t_dH.tensor, offset=hofs + 16 - 8 * qt,
                                            ap=[[16 * 768 - 1, 8], [768, 16], [1, 744]])
                            nc.scalar.dma_start(out=dstH1, in_=t_sb[:, :])
                            dstW1 = bass.AP(tensor=t_dW.tensor, offset=wofs + 16,
                                            ap=[[16 * 768, 8], [768 - 1, 16], [1, 744]])
                            nc.scalar.dma_start(out=dstW1, in_=t_sb[:, :])
                            # --- plain gathers -> t_s [128, (h, e32)] ---
                            t_s = p2b.tile([P, NH * 32], BF16, tag="ts", bufs=3)
                            srcH2 = bass.AP(tensor=t_dH.tensor, offset=hofs + 16,
                                            ap=[[768, P], [62, NH], [1, 16]])
                            dstH = bass.AP(tensor=t_s.tensor, offset=t_s[:, :].offset,
                                           ap=[[t_s.tensor.shape[1], P], [32, NH], [1, 16]])
                            nc.sync.dma_start(out=dstH, in_=srcH2)
                            srcW2 = bass.AP(tensor=t_dW.tensor, offset=wofs + 16 + 31,
                                            ap=[[768, P], [62, NH], [1, 16]])
                            dstW = bass.AP(tensor=t_s.tensor, offset=t_s[:, :].offset + 16,
                                           ap=[[t_s.tensor.shape[1], P], [32, NH], [1, 16]])
                            nc.sync.dma_start(out=dstW, in_=srcW2)
                            # --- batched transposes: tb [96(3h,32e), (b4, 128q)] ---
                            tb_ps = psp.tile([96, 512], BF16, tag="s", bufs=2)
                            for b4 in range(4):
                                nc.tensor.transpose(tb_ps[:, b4 * P:(b4 + 1) * P],
                                                    t_s[:, b4 * 96:(b4 + 1) * 96], ident[:, :])
                            tb_sb = p2b.tile([96, 512], BF16, tag="tb", bufs=3)
                            nc.vector.tensor_copy(tb_sb[:, :], tb_ps[:, :])
                            # --- S^T blocks + exp -> pT [128k, (h, kc, 128q)] ---
                            pT = p2b.tile([P, NH * NT], BF16, tag="pT", bufs=2)
                            for g in range(6):
                                s_ps = psp.tile([P, 512], FP32, tag="s", bufs=2)
                                for hh in range(2):
                                    h = 2 * g + hh
                                    hb = (h % 3) * 32
                                    bb = h // 3
                                    for kc in range(2):
                                        blk = slice((2 * hh + kc) * P, (2 * hh + kc + 1) * P)
                                        nc.tensor.matmul(s_ps[:, blk],
                                                         k_sb[:, h * NT + kc * P:h * NT + (kc + 1) * P],
                                                         q_sb[:, h * P:(h + 1) * P],
                                                         start=True, stop=False)
                                        nc.tensor.matmul(s_ps[:, blk],
                                                         indic_sb[hb:hb + 32, kc * P:(kc + 1) * P],
                                                         tb_sb[hb:hb + 32, bb * P:(bb + 1) * P],
                                                         start=False, stop=True)
                                nc.scalar.activation(pT[:, g * 512:(g + 1) * 512],
                                                     s_ps[:, :], AF.Exp)
                            # --- PV (token-major) with ones-column denominators ---
                            o_ps = psp.tile([P, NH, 65], FP32, tag="o", bufs=1)
                            for h in range(NH):
                                for kc in range(2):
                                    nc.tensor.matmul(o_ps[:, h, :],
                                                     pT[:, h * NT + kc * P:h * NT + (kc + 1) * P],
                                                     v_sb[kc][:, h * 65:(h + 1) * 65],
                                                     start=(kc == 0), stop=(kc == 1))
                            recc = p2b.tile([P, NH], FP32, tag="recc", bufs=2)
                            nc.vector.reciprocal(recc[:, :], o_ps[:, :, 64])
                            attn_tok = p2b.tile([P, DIM], BF16, tag="atok", bufs=2)
                            for h in range(NH):
                                nc.vector.tensor_scalar(attn_tok[:, h * HD:(h + 1) * HD],
                                                        o_ps[:, h, 0:HD],
                                                        recc[:, h:h + 1], None, op0=ALU.mult)
                            # --- transpose to channel-major + proj ---
                            tr_ps = psp.tile([P, DIM], BF16, tag="big", bufs=2)
                            for j in range(6):
                                nc.tensor.transpose(tr_ps[:, j * P:(j + 1) * P],
                                                    attn_tok[:, j * P:(j + 1) * P], ident[:, :])
                            attnT = p2b.tile([P, DIM], BF16, tag="atok", bufs=2)
                            nc.vector.tensor_copy(attnT[:, :], tr_ps[:, :])
                            pr_ps = psp.tile([P, DIM], FP32, tag="big", bufs=2)
                            for n2, nsl in ((0, slice(0, 512)), (1, slice(512, 768))):
                                for j in range(6):
                                    nc.tensor.matmul(pr_ps[:, nsl], attnT[:, j * P:(j + 1) * P],
                                                     wp_sb[j][:, nsl], start=(j == 0), stop=False)
                                nc.tensor.matmul(pr_ps[:, nsl], ones1[:, :], bp_sb[:, nsl],
                                                 start=False, stop=True)
                            # --- residual + shuffle-adapter + inline LN2 ---
                            tglob = w * 2 + qt
                            tsl = slice(tglob * P, (tglob + 1) * P)
                            xt = p2b.tile([P, DIM], FP32, tag="xres", bufs=2)
                            nc.sync.dma_start(out=xt[:, :], in_=x_in[tsl, :])
                            adt = p2b.tile([P, DIM], BF16, tag="adt", bufs=2)
                            nc.sync.dma_start(out=adt[:, :], in_=ad_d[:, tsl], transpose=True)
                            nc.vector.tensor_tensor(xt[:, :], xt[:, :], pr_ps[:, :], op=ALU.add)
                            xm = xmp.tile([P, DIM], BF16, tag="xm", bufs=6)
                            ad_shuf = bass.AP(tensor=adt.tensor, offset=adt[:, :].offset,
                                              ap=[[adt.tensor.shape[1], P], [1, 384], [384, 2]])
                            xm_v = bass.AP(tensor=xm.tensor, offset=xm[:, :].offset,
                                           ap=[[xm.tensor.shape[1], P], [2, 384], [1, 2]])
                            xt_v = bass.AP(tensor=xt.tensor, offset=xt[:, :].offset,
                                           ap=[[xt.tensor.shape[1], P], [2, 384], [1, 2]])
                            nc.vector.scalar_tensor_tensor(xm_v, ad_shuf, BLOCK_SCALE, xt_v,
                                                           op0=ALU.mult, op1=ALU.add)
                            xm_tiles.append(xm)
                            sm2 = p2b.tile([P, 1], FP32, tag="sm2", bufs=2)
                            nc.vector.tensor_reduce(sm2[:, :], xm[:, :],
                                                    axis=mybir.AxisListType.X, op=ALU.add)
                            scr2 = p2b.tile([P, DIM], BF16, tag="scr2", bufs=2)
                            sq2 = p2b.tile([P, 1], FP32, tag="sq2", bufs=2)
                            nc.scalar.activation(scr2[:, :], xm[:, :], AF.Square,
                                                 accum_out=sq2[:, :])
                            mean2 = p2b.tile([P, 1], FP32, tag="mean2", bufs=2)
                            nc.vector.tensor_scalar(mean2[:, :], sm2[:, :], 1.0 / DIM, None, op0=ALU.mult)
                            var2 = p2b.tile([P, 1], FP32, tag="var2", bufs=2)
                            nc.vector.tensor_scalar(var2[:, :], sq2[:, :], 1.0 / DIM, None, op0=ALU.mult)
                            m2c2 = p2b.tile([P, 1], FP32, tag="m2c2", bufs=2)
                            nc.vector.tensor_tensor(m2c2[:, :], mean2[:, :], mean2[:, :], op=ALU.mult)
                            nc.vector.tensor_tensor(var2[:, :], var2[:, :], m2c2[:, :], op=ALU.subtract)
                            sdv2 = p2b.tile([P, 1], FP32, tag="sdv2", bufs=2)
                            nc.scalar.activation(sdv2[:, :], var2[:, :], AF.Ln, bias=eps_col[:, :])
                            rstd2 = p2b.tile([P, 1], FP32, tag="rstd2", bufs=2)
                            nc.scalar.activation(rstd2[:, :], sdv2[:, :], AF.Exp, scale=-0.5)
                            n2t = p2b.tile([P, DIM], BF16, tag="scr2", bufs=2)
                            nc.vector.tensor_scalar(n2t[:, :], xm[:, :], mean2[:, :],
                                                    rstd2[:, :], op0=ALU.subtract, op1=ALU.mult)
                            nc.scalar.dma_start(out=norm2_d[tsl, :], in_=n2t[:, :])
                    # ---- MLP for window pair (emitted after odd windows) ----
                    if w % 2 == 0:
                        continue
                    for wm in (w - 1, w):
                        wsl = slice(wm * NT, (wm + 1) * NT)
                        ntc2 = [ntcp.tile([P, NT], BF16, tag="ntc", bufs=6, name=f"n2T{wm}_{_i}") for _i in range(6)]
                        for k in range(6):
                            nc.sync.dma_start(out=ntc2[k][:, :],
                                              in_=norm2_d[wsl, k * P:(k + 1) * P], transpose=True)
                        hT = [h5.tile([P, NT], BF16, tag="hT", bufs=25,
                                      name=f"hT{wm}_{_i}") for _i in range(24)]
                        for m in range(24):
                            ps = psp.tile([P, NT], FP32, tag="s", bufs=2)
                            for k in range(6):
                                nc.tensor.matmul(ps[:, :], wm1_sb[k][:, m * P:(m + 1) * P],
                                                 ntc2[k][:, :], start=(k == 0), stop=(k == 5))
                            nc.scalar.activation(hT[m][:, :], ps[:, :], AF.Gelu,
                                                 bias=bm1_sb[:, m:m + 1])
                        for tt in range(2):
                            tglob = wm * 2 + tt
                            ps = psp.tile([P, DIM], FP32, tag="big", bufs=2)
                            for n2, nsl in ((0, slice(0, 512)), (1, slice(512, 768))):
                                for k in range(24):
                                    nc.tensor.matmul(ps[:, nsl],
                                                     hT[k][:, tt * P:(tt + 1) * P],
                                                     wm2_sb[k][:, nsl],
                                                     start=(k == 0), stop=False)
                                nc.tensor.matmul(ps[:, nsl], ones1[:, :], bm2_sb[:, nsl],
                                                 start=False, stop=True)
                            ot = p2b.tile([P, DIM], FP32, tag="xres", bufs=2)
                            nc.vector.tensor_tensor(ot[:, :], ps[:, :],
                                                    xm_tiles[tglob][:, :], op=ALU.add)
                            nc.scalar.dma_start(out=out_ext[tglob * P:(tglob + 1) * P, :],
                                                in_=ot[:, :])
            if KDBG:
                for (srcten, dstten, rows) in ((q_d, dbgq, DIM), (k_d, dbgk, DIM),
                                               (v2_d, dbgv, VRP), (ad_d, dbgad, DIM)):
                    nc.sync.dma_start(out=dstten[0:rows, :], in_=srcten[0:rows, :])
                for (srcten, dstten) in ((norm_d, dbgn1), (norm2_d, dbgn2)):
                    nc.sync.dma_start(out=dstten[0:T, :], in_=srcten[0:T, :])
    nc.finalize()
    return nc


_GRAPH = None


def _window_permute(x):
    # [B, H, W, D] -> [B, T, D] in window-major token order (h-major in window)
    xb = x.reshape(B, H // WS, WS, W // WS, WS, DIM).transpose(0, 1, 3, 2, 4, 5)
    return np.ascontiguousarray(xb.reshape(B, T, DIM))


def _window_unpermute(y):
    yb = y.reshape(B, H // WS, W // WS, WS, WS, DIM).transpose(0, 1, 3, 2, 4, 5)
    return np.ascontiguousarray(yb.reshape(B, H, W, DIM))


def kernel(x, w1, b1, Wqkv, bqkv, A1, ba1, A2, ba2, aw, rel_h, rel_w, Wp, bp,
           cw, cb, sw, sb, w2, b2, Wm1, bm1, Wm2, bm2):
    global _GRAPH
    x = np.asarray(x, np.float32)
    f = lambda a: np.asarray(a, np.float32)
    w1, b1, Wqkv, bqkv = f(w1), f(b1), f(Wqkv), f(bqkv)
    A1, ba1, A2, ba2 = f(A1), f(ba1), f(A2), f(ba2)
    aw = float(np.asarray(aw))
    rel_h, rel_w, Wp, bp = f(rel_h), f(rel_w), f(Wp), f(bp)
    cw, cb, sw, sb = f(cw).ravel(), f(cb).ravel(), f(sw).ravel(), f(sb).ravel()
    w2, b2, Wm1, bm1, Wm2, bm2 = f(w2), f(b2), f(Wm1), f(bm1), f(Wm2), f(bm2)

    # ---- host weight folds ----
    Wqkv_f = w1[:, None] * Wqkv
    bqkv_f = b1 @ Wqkv + bqkv
    ksl = slice(DIM, 2 * DIM)
    Wqkv_f[:, ksl] *= SCALE
    bqkv_k = bqkv_f.copy(); bqkv_k[ksl] *= SCALE
    A1_f = A1.copy(); A1_f[ksl, :] /= SCALE
    A2_f = aw * A2
    ba2_f = aw * ba2
    A2_f[:, ksl] *= SCALE
    ba2_k = ba2_f.copy(); ba2_k[ksl] *= SCALE
    Wm1_f = w2[:, None] * Wm1
    bm1_f = b2 @ Wm1 + bm1
    relcat = np.concatenate([rel_h, rel_w], 0)        # [62, 64]
    relcatT_np = _bf16(relcat.T)                      # [64, 62]
    indic_np = np.zeros((32, NT), np.float32)
    for j in range(16):
        for kh in range(16):
            for kw in range(16):
                if kh == 15 - j:
                    indic_np[j, kh * 16 + kw] = 1.0
                if kw == 15 - j:
                    indic_np[16 + j, kh * 16 + kw] = 1.0
    indic96_np = np.concatenate([indic_np] * 3, 0)    # [96, 256] 3x replicated

    # fp8 adapter weight packing (DoubleRow pairs along contraction dim)
    A1_p = np.zeros((3 * DIM, AD8), np.float32)
    A1_p[:, :AD] = A1_f                               # cols AD..AD8 zero
    a1w8_np = _f8(A1_p.reshape(9, 2, 128, AD8).transpose(0, 2, 1, 3)
                  .reshape(9 * 128, 2 * AD8))
    A2_p = np.zeros((AD8, 3 * DIM), np.float32)
    A2_p[:AD, :] = A2_f                               # rows AD..AD8 zero
    a2w8_np = _f8(A2_p[:512].reshape(2, 2, 128, 3 * DIM).transpose(0, 2, 1, 3)
                  .reshape(2 * 128, 2 * 3 * DIM))
    a2wl_np = _f8(A2_p[512:AD8])

    feeds = {
        "wqkv": _bf16(Wqkv_f), "a1w8": a1w8_np, "a2w8": a2w8_np, "a2wl": a2wl_np,
        "wp": _bf16(Wp), "wm1": _bf16(Wm1_f), "wm2": _bf16(Wm2),
        "relcatT": relcatT_np, "indic96": _bf16(indic96_np),
        "bqkv_c": _col_tiles(bqkv_k), "ba1_c": _col_tiles(np.pad(ba1, (0, AD8 - AD))),
        "ba2_c": _col_tiles(ba2_k), "bm1_c": _col_tiles(bm1_f),
        "bp_r": _bf16(bp.reshape(1, DIM)), "bm2_r": _bf16(bm2.reshape(1, DIM)),
        "w1_c": _col_tiles(w1), "b1_c": _col_tiles(b1),
        "w1_r": w1.reshape(1, DIM).astype(np.float32),
        "b1_r": b1.reshape(1, DIM).astype(np.float32),
        "cw_r": cw.reshape(1, HID).astype(np.float32),
        "cb_r": cb.reshape(1, HID).astype(np.float32),
        "sw_r": sw.reshape(1, HID).astype(np.float32),
        "sb_r": sb.reshape(1, HID).astype(np.float32),
    }

    xp = _window_permute(x)
    in_maps = [dict(feeds, x=np.ascontiguousarray(xp[i])) for i in range(B)]

    if _GRAPH is None:
        _GRAPH = build_graph()
    import os
    trace = os.environ.get("KTRACE", "0") == "1"
    res = run_bass_kernel_spmd(_GRAPH, in_maps, core_ids=list(range(B)), trace=trace)
    globals()['_LAST_RES'] = res
    if trace and res.exec_time_ns is not None:
        print(f"HW exec time: {res.exec_time_ns} ns")
    y = np.stack([res.results[i]["out"] for i in range(B)], 0)
    return _window_unpermute(y).astype(np.float32)

